# revision 20
# baseline (speedup 1.0000x reference)
"""DenseDilatedKnnGraph Bass kernel for TRN2 (8 NeuronCores).

Problem: x (8, 32, 4096, 1) fp32 -> edge_index (2, 8, 4096, 9) int32.
For each batch b and point i: the 9 dilated nearest neighbours
(ranks 0,2,...,16 of the top-18 smallest squared euclidean distances),
plus the broadcast center index.

Sharding: data-parallel over batch B — one batch per NeuronCore.

Per-core design (index-packed candidate retrieval + host re-rank):
  - score v'(i,j) = alpha*(p_i.p_j - |p_j|^2/2) + C0 mapped into the
    fp32 binade [128, 256). ONE bf16 matmul of contract depth 99
    (hi/lo split: hi.hi + hi.lo + lo.hi + 3-way split of the |p_j|^2
    row) gives ~2^-17 relative accuracy at 1 PE cycle/row.
  - pack pk = RN(RN(RN(v'+2^19) - 2^19) + (4095-j)*2^-16): +2^19
    quantizes to the 0.0625 grid, -2^19 is Sterbenz-exact, the idx add
    is exact; pk's low 12 mantissa bits then hold the ORIGINAL column
    id and fp32 ordering == (quantized score, ascending j). All paths
    verified bit-exact on hardware.
  - columns are host-interleaved (device col c = original j with
    j = (c%8)*512 + c//512-ish transpose) so each contiguous tooth is
    a uniform sample of original indices.
  - banks 0-2 (W, 1536 cols): PE accumulate-matmuls (+2^19; -2^19;
    +idx as two bf16 rows) fold the pack into PSUM; DVE combs ONE
    1536-wide tooth STRAIGHT FROM PSUM. No ACT, no SBUF staging.
  - banks 3-7 (Z, 2560 cols): ACT drains with +2^19 bias, an in-place
    ACT pass subtracts 2^19 from cols [0:2304] (a Pool accumulate-DMA
    covers the rest), two Pool accumulate-DMAs add idxfrac; DVE combs
    four 640-wide teeth from SBUF, software-pipelined one tile behind
    so DVE never waits on the pack chain.
  - 40 packed candidates/row. No merge, no MaxIndex: indices travel
    inside the values. Engines balance at ~148us each (DVE comb floor).
  - Host: decode indices, re-rank candidates with exact fp64 scores;
    rows where a tooth's 8th-best packed cutoff proves the set might
    miss a true member affecting even ranks (cutoff + quantum/2 + eps
    >= exact 17th best) get an exact recompute over the flagged
    tooth's columns. Output = even ranks 0,2,...,16.
"""

import numpy as np
import ml_dtypes
from contextlib import ExitStack

import concourse.bass as bass
import concourse.bacc as bacc
import concourse.mybir as mybir
from concourse.tile import TileContext
from concourse.bass_utils import run_bass_kernel_spmd

B, C, N = 8, 32, 4096
FP32 = mybir.dt.float32
BF16 = mybir.dt.bfloat16

# score mapping: v = p_i.p_j - |p_j|^2/2 lies in [-200, 95] with margin
# (measured [-187.96, 85.02] for this dataset); map to [130, 252].
ALPHA = 122.0 / 295.0
C0 = np.float32(130.0 + ALPHA * 200.0)
BIG = float(2.0 ** 19)           # quantizer; ulp(2^19) = 0.0625
QUANT = 0.0625
EPS = 0.01                       # bf16-matmul error margin for the row flags
KALL = 18

# device col c = a*512 + b holds original point j = b*8 + a
PERM = (np.arange(N).reshape(512, 8).T).ravel()
WCOLS = 1536                     # banks 0-2: PE-packed, combed from PSUM
ZCOLS = N - WCOLS                # banks 3-7: ACT/Pool-packed, combed from SBUF
TEETH = [(0, 1536),
         (1536, 2176), (2176, 2816), (2816, 3456), (3456, 4096)]
NCAND = 8 * len(TEETH)


def _emit(tc, lr_in, idxf_in, acc_in, onn):
    nc = tc.nc
    with ExitStack() as ctx:
        const = ctx.enter_context(tc.tile_pool(name="const", bufs=1))
        psa_pool = ctx.enter_context(tc.tile_pool(name="psa", bufs=1, space="PSUM"))
        psb_pool = ctx.enter_context(tc.tile_pool(name="psb", bufs=1, space="PSUM"))
        tpool = ctx.enter_context(tc.tile_pool(name="t", bufs=4))
        cpool = ctx.enter_context(tc.tile_pool(name="cand", bufs=5))

        lhs = const.tile([99, N], BF16)    # [hi;hi;lo;1;1;1]
        rhs = const.tile([99, N], BF16)    # [hi;lo;hi;r1;r2;r3]
        idxf = const.tile([128, ZCOLS], FP32)  # (4095-j)*2^-16, Z cols only
        acp = const.tile([1, WCOLS], BF16)     # +2^19        (W cols)
        acm = const.tile([1, WCOLS], BF16)     # -2^19        (W cols)
        aci = const.tile([2, WCOLS], BF16)     # [khi;klo]    (W cols)
        ones = const.tile([2, 128], BF16)
        nbig = const.tile([128, 256], FP32)    # -2^19, Pool's share of U

        # startup: tile 0's W-matmuls need lhs block 0, rhs bank 0 and the
        # accumulate rows; spread uploads over engines that are idle anyway
        nc.gpsimd.memset(ones[:, :], 1.0)
        nc.gpsimd.memset(acp[:, :], BIG)
        nc.gpsimd.memset(acm[:, :], -BIG)
        nc.gpsimd.memset(nbig[:, :], -BIG)
        nc.sync.dma_start(out=lhs[:, 0:512], in_=lr_in[0:99, 0:512])
        nc.sync.dma_start(out=rhs[:, 0:512], in_=lr_in[99:198, 0:512])
        nc.sync.dma_start(out=aci[:, :], in_=acc_in[2:4, :])
        for q in range(2):
            sl = slice(q * 1280, (q + 1) * 1280)
            nc.gpsimd.dma_start(out=idxf[:, sl], in_=idxf_in[:, sl])
        for nq in range(1, 8):
            sl = slice(nq * 512, (nq + 1) * 512)
            nc.sync.dma_start(out=rhs[:, sl], in_=lr_in[99:198, sl])
        for nq in range(1, 8):
            sl = slice(nq * 512, (nq + 1) * 512)
            nc.sync.dma_start(out=lhs[:, sl], in_=lr_in[0:99, sl])

        hist = {}     # m -> (psa, t, cand)
        SKEW = 1
        for m in range(32 + SKEW):
            if m < 32:
                ms = slice(m * 128, (m + 1) * 128)
                # --- W: banks 0-2, full pack in PSUM via accumulates ---
                psa = psa_pool.tile([128, WCOLS], FP32, tag="mma")
                for k in range(3):
                    ks = slice(k * 512, (k + 1) * 512)
                    nc.tensor.matmul(psa[:, ks], lhs[:, ms], rhs[:, ks],
                                     start=True, stop=False)
                    nc.tensor.matmul(psa[:, ks], ones[0:1, :], acp[:, ks],
                                     start=False, stop=False)
                    nc.tensor.matmul(psa[:, ks], ones[0:1, :], acm[:, ks],
                                     start=False, stop=False)
                    nc.tensor.matmul(psa[:, ks], ones[:, :], aci[:, ks],
                                     start=False, stop=True)
                # --- Z: banks 3-7 ---
                psb = psb_pool.tile([128, ZCOLS], FP32, tag="mmb")
                for k in range(5):
                    ks = slice(k * 512, (k + 1) * 512)
                    nc.tensor.matmul(psb[:, ks], lhs[:, ms],
                                     rhs[:, WCOLS + k * 512:WCOLS + (k + 1) * 512],
                                     start=True, stop=True)
                t = tpool.tile([128, ZCOLS], FP32)
                nc.scalar.activation(t[:, :], psb[:, :],
                                     mybir.ActivationFunctionType.Copy, bias=BIG)
                # unshift: ACT takes cols [0:2304], Pool adds -2^19 to the rest
                nc.scalar.activation(t[:, 0:2304], t[:, 0:2304],
                                     mybir.ActivationFunctionType.Copy, bias=-BIG)
                nc.gpsimd.dma_start(out=t[:, 2304:2560], in_=nbig[:, :],
                                    accum_op=mybir.AluOpType.add)
                # NB: a single accumulate-DMA is only correct up to 8KB per
                # partition (2048 fp32); split 2560 cols into two chunks
                for h in range(2):
                    hs = slice(h * 1280, (h + 1) * 1280)
                    nc.gpsimd.dma_start(out=t[:, hs], in_=idxf[:, hs],
                                        accum_op=mybir.AluOpType.add)
                cand = cpool.tile([128, NCAND], FP32)
            # --- combs: Z of tile m-SKEW first (ready), then W of tile m ---
            if m >= SKEW:
                mp = m - SKEW
                psa_p, t_p, cand_p = hist.pop(mp)
                for z in range(4):
                    c0, c1 = TEETH[1 + z]
                    nc.vector.max(out=cand_p[:, (1 + z) * 8:(2 + z) * 8],
                                  in_=t_p[:, c0 - WCOLS:c1 - WCOLS])
            if m < 32:
                nc.vector.max(out=cand[:, 0:8], in_=psa[:, 0:WCOLS])
            if m >= SKEW:
                nc.sync.dma_start(out=onn[mp * 128:(mp + 1) * 128, :],
                                  in_=cand_p[:, :])
            if m < 32:
                hist[m] = (psa, t, cand)


_NC_CACHE = {}


def _get_nc():
    if "nc" not in _NC_CACHE:
        nc = bacc.Bacc()
        lr = nc.declare_dram_parameter("lr", [198, N], BF16, isOutput=False)
        idxf = nc.declare_dram_parameter("idxf", [128, ZCOLS], FP32, isOutput=False)
        acc = nc.declare_dram_parameter("acc", [4, WCOLS], BF16, isOutput=False)
        onn = nc.declare_dram_parameter("nn", [N, NCAND], FP32, isOutput=True)
        with TileContext(nc) as tc:
            _emit(tc, lr, idxf, acc, onn)
        nc.finalize()
        _NC_CACHE["nc"] = nc
    return _NC_CACHE["nc"]


def _prep(xb):
    """Per-batch host prep: xb (C, N) fp32 -> lr rows (198, N) bf16,
    columns in device (interleaved) order."""
    ps = (np.ascontiguousarray(xb).T.astype(np.float64)[PERM]
          * np.sqrt(ALPHA))                           # (N, C), device order
    psf = ps.astype(np.float32)
    hi = psf.astype(ml_dtypes.bfloat16)
    lo = (psf - hi.astype(np.float32)).astype(ml_dtypes.bfloat16)
    sqs = np.einsum("nc,nc->n", ps, ps)
    row = float(C0) - 0.5 * sqs                       # fp64
    r1 = row.astype(ml_dtypes.bfloat16)
    row2 = row - r1.astype(np.float64)
    r2 = row2.astype(ml_dtypes.bfloat16)
    r3 = (row2 - r2.astype(np.float64)).astype(ml_dtypes.bfloat16)
    one = np.ones((1, N), ml_dtypes.bfloat16)
    hiT, loT = hi.T, lo.T                             # (C, N)
    lr = np.concatenate([
        hiT, hiT, loT, one, one, one,                 # lhs rows
        hiT, loT, hiT, r1[None, :], r2[None, :], r3[None, :],  # rhs rows
    ], axis=0)
    assert lr.shape == (198, N)
    return np.ascontiguousarray(lr)


_CONSTS = {}


def _get_consts():
    if not _CONSTS:
        k = (N - 1 - PERM).astype(np.int64)           # idx payload per device col
        fracz = (k[WCOLS:] * 2.0 ** -16).astype(np.float32)
        _CONSTS["idxf"] = np.ascontiguousarray(np.broadcast_to(fracz, (128, ZCOLS)))
        kw = k[:WCOLS]
        khi = (kw >> 4).astype(np.float64) * 2.0 ** -12
        klo = (kw & 15).astype(np.float64) * 2.0 ** -16
        acc = np.stack([np.full(WCOLS, BIG), np.full(WCOLS, -BIG), khi, klo])
        _CONSTS["acc"] = acc.astype(ml_dtypes.bfloat16)
        assert (_CONSTS["acc"][2].astype(np.float64) == khi).all()
        assert (_CONSTS["acc"][3].astype(np.float64) == klo).all()
        inv = np.empty(N, np.int64)
        inv[PERM] = np.arange(N)
        _CONSTS["inv"] = inv
    return _CONSTS


def _run(x, trace=False, **kw):
    nc = _get_nc()
    cst = _get_consts()
    in_maps = []
    for b in range(B):
        in_maps.append({"lr": _prep(x[b, :, :, 0]), "idxf": cst["idxf"],
                        "acc": cst["acc"]})
    return run_bass_kernel_spmd(nc, in_maps, list(range(B)), trace=trace, **kw)


def _postprocess(x, cand_all):
    """Decode packed candidates, exact re-rank, flagged-tooth fallback."""
    cst = _get_consts()
    inv = cst["inv"]
    out = np.zeros((B, N, 9), np.int32)
    nt = len(TEETH)
    for b in range(B):
        cand = cand_all[b][inv]                       # rows now = original ids
        cu = cand.view(np.uint32)
        cidx = (N - 1) - (cu & 0xFFF).astype(np.int64)
        np.clip(cidx, 0, N - 1, out=cidx)
        pts = x[b, :, :, 0].T.astype(np.float64)      # (N, C) original order
        sqe = np.einsum("nc,nc->n", pts, pts)
        g = pts[cidx]
        ve = ALPHA * (np.einsum("nc,nkc->nk", pts, g) - 0.5 * sqe[cidx]) + float(C0)
        ordk = np.lexsort((cidx, -ve), axis=1)[:, :KALL]
        top = np.take_along_axis(cidx, ordk, axis=1)
        sel = top[:, 0:17:2].astype(np.int32)
        # flags: tooth cutoff could hide a member affecting even ranks
        w17 = np.take_along_axis(ve, ordk, axis=1)[:, KALL - 2]
        cuts = np.ascontiguousarray(cand.reshape(N, nt, 8)[:, :, 7])
        cutsu = cuts.view(np.uint32).reshape(N, nt)
        qv = cuts.astype(np.float64) - (cutsu & 0xFFF).astype(np.float64) * 2.0 ** -16
        flagm = (qv + (QUANT / 2 + EPS)) >= w17[:, None]     # (N, nt)
        nflag = flagm.sum(axis=1)
        multi = np.where(nflag >= 2)[0]
        if multi.size:
            vf = pts[multi] @ pts.T - 0.5 * sqe[None, :]
            part = np.argpartition(-vf, KALL + 8, axis=1)[:, :KALL + 8]
            pv = np.take_along_axis(vf, part, axis=1)
            o = np.lexsort((part, -pv), axis=1)[:, :KALL]
            sel[multi] = np.take_along_axis(part, o, axis=1)[:, 0:17:2].astype(np.int32)
        for tth in range(nt):
            rows = np.where(flagm[:, tth] & (nflag == 1))[0]
            if not rows.size:
                continue
            cols = PERM[TEETH[tth][0]:TEETH[tth][1]]  # original ids in this tooth
            vt = ALPHA * (pts[rows] @ pts[cols].T - 0.5 * sqe[cols][None, :]) + float(C0)
            allv = np.concatenate([ve[rows], vt], axis=1)
            alli = np.concatenate([cidx[rows],
                                   np.broadcast_to(cols, (rows.size, cols.size))], axis=1)
            part = np.argpartition(-allv, 2 * KALL, axis=1)[:, :2 * KALL]
            pv = np.take_along_axis(allv, part, axis=1)
            pi = np.take_along_axis(alli, part, axis=1)
            o = np.lexsort((pi, -pv), axis=1)
            si = np.take_along_axis(pi, o, axis=1)
            sv = np.take_along_axis(pv, o, axis=1)
            # drop duplicate (value,idx) pairs (tooth cols overlap candidates)
            dup = np.zeros_like(si, bool)
            dup[:, 1:] = (si[:, 1:] == si[:, :-1]) & (sv[:, 1:] == sv[:, :-1])
            res = np.full((rows.size, KALL), -1, np.int64)
            for r in range(rows.size):
                keep = si[r][~dup[r]][:KALL]
                res[r, :keep.size] = keep
            sel[rows] = res[:, 0:17:2].astype(np.int32)
        out[b] = sel
    return out


def kernel(x):
    x = np.asarray(x)
    assert x.shape == (B, C, N, 1), x.shape
    res = _run(x)
    cand_all = np.stack([np.asarray(res.results[i]["nn"], np.float32)
                         for i in range(B)])          # (B, N, NCAND)
    nn_sel = _postprocess(x, cand_all)
    center = np.broadcast_to(np.arange(N, dtype=np.int32)[None, :, None],
                             nn_sel.shape)
    return np.stack([nn_sel, center], axis=0)         # (2, B, N, 9) int32


# revision 28
# speedup vs baseline: 1.0139x; 1.0139x over previous
"""DenseDilatedKnnGraph Bass kernel for TRN2 (8 NeuronCores).

Problem: x (8, 32, 4096, 1) fp32 -> edge_index (2, 8, 4096, 9) int32.
For each batch b and point i: the 9 dilated nearest neighbours
(ranks 0,2,...,16 of the top-18 smallest squared euclidean distances),
plus the broadcast center index.

Sharding: data-parallel over batch B — one batch per NeuronCore.

Per-core design (index-packed candidate retrieval + host re-rank):
  - score v'(i,j) = alpha*(p_i.p_j - |p_j|^2/2) + C0 mapped into the
    fp32 binade [128, 256). ONE bf16 matmul of contract depth 99
    (hi/lo split: hi.hi + hi.lo + lo.hi + 3-way split of the |p_j|^2
    row) gives ~2^-17 relative accuracy at 1 PE cycle/row.
  - pack pk = RN(RN(RN(v'+2^19) - 2^19) + (4095-j)*2^-16): +2^19
    quantizes to the 0.0625 grid, -2^19 is Sterbenz-exact, the idx add
    is exact; pk's low 12 mantissa bits then hold the ORIGINAL column
    id and fp32 ordering == (quantized score, ascending j). All paths
    verified bit-exact on hardware.
  - columns are host-interleaved (device col c = original j with
    j = (c%8)*512 + c//512-ish transpose) so each contiguous tooth is
    a uniform sample of original indices.
  - banks 0-2 (W, 1536 cols): PE accumulate-matmuls (+2^19; -2^19;
    +idx as two bf16 rows) fold the pack into PSUM; DVE combs ONE
    1536-wide tooth STRAIGHT FROM PSUM. No ACT, no SBUF staging.
  - banks 3-7 (Z, 2560 cols): ACT drains with +2^19 bias, an in-place
    ACT pass subtracts 2^19 from cols [0:2304] (a Pool accumulate-DMA
    covers the rest), two Pool accumulate-DMAs add idxfrac; DVE combs
    four 640-wide teeth from SBUF, software-pipelined one tile behind
    so DVE never waits on the pack chain.
  - 40 packed candidates/row. No merge, no MaxIndex: indices travel
    inside the values. Engines balance at ~148us each (DVE comb floor).
  - Host: decode indices, re-rank candidates with exact fp64 scores;
    rows where a tooth's 8th-best packed cutoff proves the set might
    miss a true member affecting even ranks (cutoff + quantum/2 + eps
    >= exact 17th best) get an exact recompute over the flagged
    tooth's columns. Output = even ranks 0,2,...,16.
"""

import numpy as np
import ml_dtypes
from contextlib import ExitStack

import concourse.bass as bass
import concourse.bacc as bacc
import concourse.mybir as mybir
from concourse.tile import TileContext
from concourse.bass_utils import run_bass_kernel_spmd

B, C, N = 8, 32, 4096
FP32 = mybir.dt.float32
BF16 = mybir.dt.bfloat16

# score mapping: v = p_i.p_j - |p_j|^2/2 lies in [-200, 95] with margin
# (measured [-187.96, 85.02] for this dataset); map to [130, 252].
ALPHA = 122.0 / 295.0
C0 = np.float32(130.0 + ALPHA * 200.0)
BIG = float(2.0 ** 19)           # quantizer; ulp(2^19) = 0.0625
QUANT = 0.0625
EPS = 0.01                       # bf16-matmul error margin for the row flags
KALL = 18

# device col c = a*512 + b holds original point j = b*8 + a
PERM = (np.arange(N).reshape(512, 8).T).ravel()
WCOLS = 1536                     # banks 0-2: PE-packed, combed from PSUM
ZCOLS = N - WCOLS                # banks 3-7: ACT/Pool-packed, combed from SBUF
TEETH = [(0, 1536),
         (1536, 2389), (2389, 3242), (3242, 4096)]
NCAND = 8 * len(TEETH)


def _emit(tc, lr_in, idxf_in, acc_in, onn):
    nc = tc.nc
    with ExitStack() as ctx:
        const = ctx.enter_context(tc.tile_pool(name="const", bufs=1))
        psa_pool = ctx.enter_context(tc.tile_pool(name="psa", bufs=1, space="PSUM"))
        psb_pool = ctx.enter_context(tc.tile_pool(name="psb", bufs=1, space="PSUM"))
        tpool = ctx.enter_context(tc.tile_pool(name="t", bufs=4))
        cpool = ctx.enter_context(tc.tile_pool(name="cand", bufs=5))

        lhs = const.tile([99, N], BF16)    # [hi;hi;lo;1;1;1]
        rhs = const.tile([99, N], BF16)    # [hi;lo;hi;r1;r2;r3]
        idxf = const.tile([128, ZCOLS], FP32)  # (4095-j)*2^-16, Z cols only
        acp = const.tile([1, WCOLS], BF16)     # +2^19        (W cols)
        acm = const.tile([1, WCOLS], BF16)     # -2^19        (W cols)
        aci = const.tile([2, WCOLS], BF16)     # [khi;klo]    (W cols)
        ones = const.tile([2, 128], BF16)
        nbig = const.tile([128, 256], FP32)    # -2^19, Pool's share of U

        # startup: tile 0's W-matmuls need lhs block 0, rhs bank 0 and the
        # accumulate rows; spread uploads over engines that are idle anyway
        nc.gpsimd.memset(ones[:, :], 1.0)
        nc.gpsimd.memset(acp[:, :], BIG)
        nc.gpsimd.memset(acm[:, :], -BIG)
        nc.gpsimd.memset(nbig[:, :], -BIG)
        nc.sync.dma_start(out=lhs[:, 0:512], in_=lr_in[0:99, 0:512])
        nc.sync.dma_start(out=rhs[:, 0:512], in_=lr_in[99:198, 0:512])
        nc.sync.dma_start(out=aci[:, :], in_=acc_in[2:4, :])
        for q in range(2):
            sl = slice(q * 1280, (q + 1) * 1280)
            nc.gpsimd.dma_start(out=idxf[:, sl], in_=idxf_in[:, sl])
        for nq in (5, 6, 7):
            sl = slice(nq * 512, (nq + 1) * 512)
            nc.scalar.dma_start(out=rhs[:, sl], in_=lr_in[99:198, sl])
        for nq in (1, 2, 3, 4):
            sl = slice(nq * 512, (nq + 1) * 512)
            nc.sync.dma_start(out=rhs[:, sl], in_=lr_in[99:198, sl])
        for nq in range(1, 8):
            sl = slice(nq * 512, (nq + 1) * 512)
            nc.sync.dma_start(out=lhs[:, sl], in_=lr_in[0:99, sl])

        hist = {}     # m -> (psa, t, cand)
        SKEW = 1
        for m in range(32 + SKEW):
            if m < 32:
                ms = slice(m * 128, (m + 1) * 128)
                # --- W: banks 0-2, full pack in PSUM via accumulates ---
                psa = psa_pool.tile([128, WCOLS], FP32, tag="mma")
                for k in range(3):
                    ks = slice(k * 512, (k + 1) * 512)
                    nc.tensor.matmul(psa[:, ks], lhs[:, ms], rhs[:, ks],
                                     start=True, stop=False)
                    nc.tensor.matmul(psa[:, ks], ones[0:1, :], acp[:, ks],
                                     start=False, stop=False)
                    nc.tensor.matmul(psa[:, ks], ones[0:1, :], acm[:, ks],
                                     start=False, stop=False)
                    nc.tensor.matmul(psa[:, ks], ones[:, :], aci[:, ks],
                                     start=False, stop=True)
                # --- Z: banks 3-7 ---
                psb = psb_pool.tile([128, ZCOLS], FP32, tag="mmb")
                for k in range(5):
                    ks = slice(k * 512, (k + 1) * 512)
                    nc.tensor.matmul(psb[:, ks], lhs[:, ms],
                                     rhs[:, WCOLS + k * 512:WCOLS + (k + 1) * 512],
                                     start=True, stop=True)
                t = tpool.tile([128, ZCOLS], FP32)
                nc.scalar.activation(t[:, :], psb[:, :],
                                     mybir.ActivationFunctionType.Copy, bias=BIG)
                # unshift: ACT cols [0:2304], Pool adds -2^19 to the rest
                nc.scalar.activation(t[:, 0:2304], t[:, 0:2304],
                                     mybir.ActivationFunctionType.Copy, bias=-BIG)
                nc.gpsimd.dma_start(out=t[:, 2304:2560], in_=nbig[:, :],
                                    accum_op=mybir.AluOpType.add)
                # NB: a single accumulate-DMA is only correct up to 8KB per
                # partition (2048 fp32); split 2560 cols into two chunks
                for h in range(2):
                    hs = slice(h * 1280, (h + 1) * 1280)
                    nc.gpsimd.dma_start(out=t[:, hs], in_=idxf[:, hs],
                                        accum_op=mybir.AluOpType.add)
                cand = cpool.tile([128, NCAND], FP32)
            # --- combs: Z of tile m-SKEW first (ready), then W of tile m ---
            if m >= SKEW:
                mp = m - SKEW
                psa_p, t_p, cand_p = hist.pop(mp)
                for z in range(len(TEETH) - 1):
                    c0, c1 = TEETH[1 + z]
                    nc.vector.max(out=cand_p[:, (1 + z) * 8:(2 + z) * 8],
                                  in_=t_p[:, c0 - WCOLS:c1 - WCOLS])
            if m < 32:
                nc.vector.max(out=cand[:, 0:8], in_=psa[:, 0:WCOLS])
            if m >= SKEW:
                nc.sync.dma_start(out=onn[mp * 128:(mp + 1) * 128, :],
                                  in_=cand_p[:, :])
            if m < 32:
                hist[m] = (psa, t, cand)


_NC_CACHE = {}


def _get_nc():
    if "nc" not in _NC_CACHE:
        nc = bacc.Bacc()
        lr = nc.declare_dram_parameter("lr", [198, N], BF16, isOutput=False)
        idxf = nc.declare_dram_parameter("idxf", [128, ZCOLS], FP32, isOutput=False)
        acc = nc.declare_dram_parameter("acc", [4, WCOLS], BF16, isOutput=False)
        onn = nc.declare_dram_parameter("nn", [N, NCAND], FP32, isOutput=True)
        with TileContext(nc) as tc:
            _emit(tc, lr, idxf, acc, onn)
        nc.finalize()
        _NC_CACHE["nc"] = nc
    return _NC_CACHE["nc"]


def _prep(xb):
    """Per-batch host prep: xb (C, N) fp32 -> lr rows (198, N) bf16,
    columns in device (interleaved) order."""
    ps = (np.ascontiguousarray(xb).T.astype(np.float64)[PERM]
          * np.sqrt(ALPHA))                           # (N, C), device order
    psf = ps.astype(np.float32)
    hi = psf.astype(ml_dtypes.bfloat16)
    lo = (psf - hi.astype(np.float32)).astype(ml_dtypes.bfloat16)
    sqs = np.einsum("nc,nc->n", ps, ps)
    row = float(C0) - 0.5 * sqs                       # fp64
    r1 = row.astype(ml_dtypes.bfloat16)
    row2 = row - r1.astype(np.float64)
    r2 = row2.astype(ml_dtypes.bfloat16)
    r3 = (row2 - r2.astype(np.float64)).astype(ml_dtypes.bfloat16)
    one = np.ones((1, N), ml_dtypes.bfloat16)
    hiT, loT = hi.T, lo.T                             # (C, N)
    lr = np.concatenate([
        hiT, hiT, loT, one, one, one,                 # lhs rows
        hiT, loT, hiT, r1[None, :], r2[None, :], r3[None, :],  # rhs rows
    ], axis=0)
    assert lr.shape == (198, N)
    return np.ascontiguousarray(lr)


_CONSTS = {}


def _get_consts():
    if not _CONSTS:
        k = (N - 1 - PERM).astype(np.int64)           # idx payload per device col
        fracz = (k[WCOLS:] * 2.0 ** -16).astype(np.float32)
        _CONSTS["idxf"] = np.ascontiguousarray(np.broadcast_to(fracz, (128, ZCOLS)))
        kw = k[:WCOLS]
        khi = (kw >> 4).astype(np.float64) * 2.0 ** -12
        klo = (kw & 15).astype(np.float64) * 2.0 ** -16
        acc = np.stack([np.full(WCOLS, BIG), np.full(WCOLS, -BIG), khi, klo])
        _CONSTS["acc"] = acc.astype(ml_dtypes.bfloat16)
        assert (_CONSTS["acc"][2].astype(np.float64) == khi).all()
        assert (_CONSTS["acc"][3].astype(np.float64) == klo).all()
        inv = np.empty(N, np.int64)
        inv[PERM] = np.arange(N)
        _CONSTS["inv"] = inv
    return _CONSTS


def _run(x, trace=False, **kw):
    nc = _get_nc()
    cst = _get_consts()
    in_maps = []
    for b in range(B):
        in_maps.append({"lr": _prep(x[b, :, :, 0]), "idxf": cst["idxf"],
                        "acc": cst["acc"]})
    return run_bass_kernel_spmd(nc, in_maps, list(range(B)), trace=trace, **kw)


def _postprocess(x, cand_all):
    """Decode packed candidates, exact re-rank, flagged-tooth fallback."""
    cst = _get_consts()
    inv = cst["inv"]
    out = np.zeros((B, N, 9), np.int32)
    nt = len(TEETH)
    for b in range(B):
        cand = cand_all[b][inv]                       # rows now = original ids
        cu = cand.view(np.uint32)
        cidx = (N - 1) - (cu & 0xFFF).astype(np.int64)
        np.clip(cidx, 0, N - 1, out=cidx)
        pts = x[b, :, :, 0].T.astype(np.float64)      # (N, C) original order
        sqe = np.einsum("nc,nc->n", pts, pts)
        g = pts[cidx]
        ve = ALPHA * (np.einsum("nc,nkc->nk", pts, g) - 0.5 * sqe[cidx]) + float(C0)
        ordk = np.lexsort((cidx, -ve), axis=1)[:, :KALL]
        top = np.take_along_axis(cidx, ordk, axis=1)
        sel = top[:, 0:17:2].astype(np.int32)
        # flags: tooth cutoff could hide a member affecting even ranks
        w17 = np.take_along_axis(ve, ordk, axis=1)[:, KALL - 2]
        cuts = np.ascontiguousarray(cand.reshape(N, nt, 8)[:, :, 7])
        cutsu = cuts.view(np.uint32).reshape(N, nt)
        qv = cuts.astype(np.float64) - (cutsu & 0xFFF).astype(np.float64) * 2.0 ** -16
        flagm = (qv + (QUANT / 2 + EPS)) >= w17[:, None]     # (N, nt)
        nflag = flagm.sum(axis=1)
        multi = np.where(nflag >= 2)[0]
        if multi.size:
            vf = pts[multi] @ pts.T - 0.5 * sqe[None, :]
            part = np.argpartition(-vf, KALL + 8, axis=1)[:, :KALL + 8]
            pv = np.take_along_axis(vf, part, axis=1)
            o = np.lexsort((part, -pv), axis=1)[:, :KALL]
            sel[multi] = np.take_along_axis(part, o, axis=1)[:, 0:17:2].astype(np.int32)
        for tth in range(nt):
            rows = np.where(flagm[:, tth] & (nflag == 1))[0]
            if not rows.size:
                continue
            cols = PERM[TEETH[tth][0]:TEETH[tth][1]]  # original ids in this tooth
            vt = ALPHA * (pts[rows] @ pts[cols].T - 0.5 * sqe[cols][None, :]) + float(C0)
            allv = np.concatenate([ve[rows], vt], axis=1)
            alli = np.concatenate([cidx[rows],
                                   np.broadcast_to(cols, (rows.size, cols.size))], axis=1)
            part = np.argpartition(-allv, 2 * KALL, axis=1)[:, :2 * KALL]
            pv = np.take_along_axis(allv, part, axis=1)
            pi = np.take_along_axis(alli, part, axis=1)
            o = np.lexsort((pi, -pv), axis=1)
            si = np.take_along_axis(pi, o, axis=1)
            sv = np.take_along_axis(pv, o, axis=1)
            # drop duplicate (value,idx) pairs (tooth cols overlap candidates)
            dup = np.zeros_like(si, bool)
            dup[:, 1:] = (si[:, 1:] == si[:, :-1]) & (sv[:, 1:] == sv[:, :-1])
            res = np.full((rows.size, KALL), -1, np.int64)
            for r in range(rows.size):
                keep = si[r][~dup[r]][:KALL]
                res[r, :keep.size] = keep
            sel[rows] = res[:, 0:17:2].astype(np.int32)
        out[b] = sel
    return out


def kernel(x):
    x = np.asarray(x)
    assert x.shape == (B, C, N, 1), x.shape
    res = _run(x)
    cand_all = np.stack([np.asarray(res.results[i]["nn"], np.float32)
                         for i in range(B)])          # (B, N, NCAND)
    nn_sel = _postprocess(x, cand_all)
    center = np.broadcast_to(np.arange(N, dtype=np.int32)[None, :, None],
                             nn_sel.shape)
    return np.stack([nn_sel, center], axis=0)         # (2, B, N, 9) int32
